# revision 3
# baseline (speedup 1.0000x reference)
"""VQ codebook quantizer for Trainium2, data-parallel across 8 NeuronCores.

Problem: z [4, 2048, 512] f32, codebook [8192, 512] f32 ->
  z_q = codebook[argmin_k ||z - c_k||^2]  (shape [4, 2048, 512]),
  indices [4, 2048] int32.

Strategy (per spec sharding hint): shard z rows (8192 total) across 8 cores
(1024 rows each); replicate the codebook. Per core:
  - distances via argmax of s[i,k] = z_i . c_k - 0.5||c_k||^2 computed on the
    PE array with an fp16 hi/lo split (3 fp16 matmuls == fp32-grade accuracy
    at 1 cycle/row instead of fp32's 4 cycles/row)
  - bias add + per-512-chunk top-8 (max8/max_index) on the vector engine
  - final argmax combine across chunks, gather z_q rows with indirect DMA.
"""

import numpy as np

import jax
import concourse.bacc as bacc
import concourse.bass as bass
import concourse.mybir as mybir
import concourse.tile as tile

B, L, D, K = 4, 2048, 512, 8192
NCORES = 8
M = (B * L) // NCORES      # 1024 z-rows per core
P = 128
NZT = M // P               # 8 z-tiles per core
KCH = 512                  # k-chunk width (one PSUM bank of fp32)
NKC = K // KCH             # 16 k-chunks
NDC = D // P               # 4 d-chunks (contraction)

f16 = mybir.dt.float16
f32 = mybir.dt.float32
u16 = mybir.dt.uint16
i32 = mybir.dt.int32


def _build_nc():
    nc = bacc.Bacc(None)
    # transposed z shard, fp16 hi/lo: [D, M]
    zh = nc.dram_tensor("zh", [D, M], f16, kind="ExternalInput")
    zl = nc.dram_tensor("zl", [D, M], f16, kind="ExternalInput")
    # transposed codebook, fp16 hi/lo: [D, K]
    ch = nc.dram_tensor("ch", [D, K], f16, kind="ExternalInput")
    cl = nc.dram_tensor("cl", [D, K], f16, kind="ExternalInput")
    # -0.5*||c_k||^2 broadcast to all partitions: [P, K]
    bias = nc.dram_tensor("bias", [P, K], f32, kind="ExternalInput")
    # natural codebook (gather source): [K, D]
    cb = nc.dram_tensor("cb", [K, D], f32, kind="ExternalInput")
    # per-partition iota 0..127 along free dim: [P, P]
    iota = nc.dram_tensor("iota", [P, P], f32, kind="ExternalInput")

    idx_out = nc.dram_tensor("idx_out", [M, 1], i32, kind="ExternalOutput")
    zq_out = nc.dram_tensor("zq_out", [M, D], f32, kind="ExternalOutput")

    with tile.TileContext(nc) as tc:
        with (
            tc.tile_pool(name="resident", bufs=1) as rp,
            tc.tile_pool(name="cbchunk", bufs=3) as cp,
            tc.tile_pool(name="scores", bufs=8) as sp,
            tc.tile_pool(name="small", bufs=2) as mp,
            tc.tile_pool(name="psum", bufs=8, space="PSUM") as ps,
        ):
            # ---- resident loads ----
            zh_sb = rp.tile([P, NDC * M], f16)   # d-chunk c at cols [c*M, (c+1)*M)
            zl_sb = rp.tile([P, NDC * M], f16)
            iota_sb = rp.tile([P, P], f32)
            nc.sync.dma_start(
                zh_sb[:].rearrange("p (c m) -> p c m", c=NDC),
                zh[:].rearrange("(c p) m -> p c m", p=P),
            )
            nc.sync.dma_start(
                zl_sb[:].rearrange("p (c m) -> p c m", c=NDC),
                zl[:].rearrange("(c p) m -> p c m", p=P),
            )
            nc.sync.dma_start(iota_sb[:], iota[:])

            # per-(z-tile) chunk stats, laid side by side:
            # cmax8_all[:, i*P + j*8 : +8] = top-8 scores of (tile i, chunk j)
            cmax8_all = rp.tile([P, NZT * P], f32)
            cidx8_all = rp.tile([P, NZT * P], u16)

            # ---- main loop: k-chunks outer (codebook streamed), z-tiles inner
            for j in range(NKC):
                ks = slice(j * KCH, (j + 1) * KCH)
                ch_t = cp.tile([P, NDC * KCH], f16, tag="ch")
                cl_t = cp.tile([P, NDC * KCH], f16, tag="cl")
                bias_t = cp.tile([P, KCH], f32, tag="bias")
                nc.sync.dma_start(
                    ch_t[:].rearrange("p (c n) -> p c n", c=NDC),
                    ch[:, ks].rearrange("(c p) n -> p c n", p=P),
                )
                nc.sync.dma_start(
                    cl_t[:].rearrange("p (c n) -> p c n", c=NDC),
                    cl[:, ks].rearrange("(c p) n -> p c n", p=P),
                )
                nc.sync.dma_start(bias_t[:], bias[:, ks])

                for i in range(NZT):
                    ms = slice(i * P, (i + 1) * P)
                    S = ps.tile([P, KCH], f32, tag="S")
                    nmm = 0
                    # group by stationary operand so weight loads amortize
                    for d in range(NDC):
                        zslice = slice(d * M + i * P, d * M + (i + 1) * P)
                        cslice = slice(d * KCH, (d + 1) * KCH)
                        for ct in (ch_t, cl_t):
                            nmm += 1
                            nc.tensor.matmul(
                                S[:],
                                lhsT=zh_sb[:, zslice],
                                rhs=ct[:, cslice],
                                start=(nmm == 1),
                                stop=False,
                            )
                    for d in range(NDC):
                        zslice = slice(d * M + i * P, d * M + (i + 1) * P)
                        cslice = slice(d * KCH, (d + 1) * KCH)
                        nmm += 1
                        nc.tensor.matmul(
                            S[:],
                            lhsT=zl_sb[:, zslice],
                            rhs=ch_t[:, cslice],
                            start=False,
                            stop=(nmm == 3 * NDC),
                        )
                    sc = sp.tile([P, KCH], f32, tag="sc")
                    nc.vector.tensor_tensor(
                        out=sc[:], in0=S[:], in1=bias_t[:], op=mybir.AluOpType.add
                    )
                    slot = slice(i * P + j * 8, i * P + j * 8 + 8)
                    nc.vector.max(out=cmax8_all[:, slot], in_=sc[:])
                    nc.vector.max_index(
                        out=cidx8_all[:, slot], in_max=cmax8_all[:, slot],
                        in_values=sc[:],
                    )

            # ---- per-z-tile combine + gather ----
            for i in range(NZT):
                tslot = slice(i * P, (i + 1) * P)
                g8 = mp.tile([P, 8], f32, tag="g8")
                p8 = mp.tile([P, 8], u16, tag="p8")
                nc.vector.max(out=g8[:], in_=cmax8_all[:, tslot])
                nc.vector.max_index(
                    out=p8[:], in_max=g8[:], in_values=cmax8_all[:, tslot]
                )
                p0f = mp.tile([P, 1], f32, tag="p0f")
                nc.vector.tensor_copy(p0f[:], p8[:, 0:1])
                cidxf = mp.tile([P, P], f32, tag="cidxf")
                nc.vector.tensor_copy(cidxf[:], cidx8_all[:, tslot])
                mask = mp.tile([P, P], f32, tag="mask")
                nc.vector.tensor_scalar(
                    out=mask[:], in0=iota_sb[:], scalar1=p0f[:, 0:1], scalar2=None,
                    op0=mybir.AluOpType.is_equal,
                )
                picked = mp.tile([P, P], f32, tag="picked")
                nc.vector.tensor_tensor(
                    out=picked[:], in0=mask[:], in1=cidxf[:],
                    op=mybir.AluOpType.mult,
                )
                inch = mp.tile([P, 1], f32, tag="inch")
                nc.vector.tensor_reduce(
                    out=inch[:], in_=picked[:], axis=mybir.AxisListType.X,
                    op=mybir.AluOpType.max,
                )
                # final = p0*64 + inchunk  (p0 = 8*j  ->  p0*64 = j*512)
                fin = mp.tile([P, 1], f32, tag="fin")
                nc.vector.tensor_scalar(
                    out=fin[:], in0=p0f[:], scalar1=64.0, scalar2=None,
                    op0=mybir.AluOpType.mult,
                )
                fin2 = mp.tile([P, 1], f32, tag="fin2")
                nc.vector.tensor_tensor(
                    out=fin2[:], in0=fin[:], in1=inch[:], op=mybir.AluOpType.add
                )
                idx_i32 = mp.tile([P, 1], i32, tag="idx")
                nc.vector.tensor_copy(idx_i32[:], fin2[:])

                zq_sb = mp.tile([P, D], f32, tag="zq")
                nc.gpsimd.indirect_dma_start(
                    out=zq_sb[:],
                    out_offset=None,
                    in_=cb[:],
                    in_offset=bass.IndirectOffsetOnAxis(ap=idx_i32[:, :1], axis=0),
                )
                nc.sync.dma_start(idx_out[i * P:(i + 1) * P, :], idx_i32[:])
                nc.sync.dma_start(zq_out[i * P:(i + 1) * P, :], zq_sb[:])
    nc.finalize()
    return nc


_CACHE = {}


def _get_compiled():
    """Build the Bass module and the sharded PJRT callable once per process."""
    if "fn" in _CACHE:
        return _CACHE["fn"]

    from jax.sharding import Mesh, PartitionSpec
    from jax.experimental.shard_map import shard_map
    from concourse.bass2jax import (
        _bass_exec_p, install_neuronx_cc_hook, partition_id_tensor,
    )

    nc = _build_nc()
    install_neuronx_cc_hook()

    in_names = []
    out_names = []
    out_avals = []
    for alloc in nc.m.functions[0].allocations:
        if not isinstance(alloc, mybir.MemoryLocationSet):
            continue
        name = alloc.memorylocations[0].name
        if alloc.kind == "ExternalInput":
            if nc.partition_id_tensor and name == nc.partition_id_tensor.name:
                continue
            in_names.append(name)
        elif alloc.kind == "ExternalOutput":
            out_names.append(name)
            out_avals.append(
                jax.core.ShapedArray(
                    tuple(alloc.tensor_shape), mybir.dt.np(alloc.dtype)
                )
            )
    n_params = len(in_names)
    n_outs = len(out_names)
    all_names = in_names + out_names
    if nc.partition_id_tensor is not None:
        all_names = all_names + [nc.partition_id_tensor.name]

    def _body(*args):
        operands = list(args)
        if nc.partition_id_tensor is not None:
            operands.append(partition_id_tensor())
        outs = _bass_exec_p.bind(
            *operands,
            out_avals=tuple(out_avals),
            in_names=tuple(all_names),
            out_names=tuple(out_names),
            lowering_input_output_aliases=(),
            sim_require_finite=True,
            sim_require_nnan=True,
            nc=nc,
        )
        return tuple(outs)

    devices = jax.devices()[:NCORES]
    mesh = Mesh(np.asarray(devices), ("core",))
    donate = tuple(range(n_params, n_params + n_outs))
    sharded = jax.jit(
        shard_map(
            _body, mesh=mesh,
            in_specs=(PartitionSpec("core"),) * (n_params + n_outs),
            out_specs=(PartitionSpec("core"),) * n_outs,
            check_rep=False,
        ),
        donate_argnums=donate,
        keep_unused=True,
    )
    _CACHE["fn"] = (sharded, in_names, out_names, out_avals)
    return _CACHE["fn"]


def _prep_in_maps(z, codebook):
    z = np.ascontiguousarray(np.asarray(z, dtype=np.float32))
    codebook = np.ascontiguousarray(np.asarray(codebook, dtype=np.float32))
    flat = z.reshape(B * L, D)

    cbT = np.ascontiguousarray(codebook.T)                     # [D, K]
    ch = cbT.astype(np.float16)
    cl = (cbT - ch.astype(np.float32)).astype(np.float16)
    cc = (codebook.astype(np.float64) ** 2).sum(1)
    bias = np.ascontiguousarray(
        np.broadcast_to((-0.5 * cc).astype(np.float32), (P, K))
    )
    iota = np.ascontiguousarray(
        np.broadcast_to(np.arange(P, dtype=np.float32), (P, P))
    )

    in_maps = []
    for c in range(NCORES):
        shard = flat[c * M:(c + 1) * M]                        # [M, D]
        zT = np.ascontiguousarray(shard.T)                     # [D, M]
        zh = zT.astype(np.float16)
        zl = (zT - zh.astype(np.float32)).astype(np.float16)
        in_maps.append(dict(
            zh=zh, zl=zl, ch=ch, cl=cl, bias=bias, cb=codebook, iota=iota,
        ))
    return in_maps


def _run(in_maps):
    sharded, in_names, out_names, out_avals = _get_compiled()
    concat_in = [
        np.concatenate([in_maps[c][name] for c in range(NCORES)], axis=0)
        for name in in_names
    ]
    concat_zeros = [
        np.zeros((NCORES * a.shape[0], *a.shape[1:]), a.dtype) for a in out_avals
    ]
    out_arrs = sharded(*concat_in, *concat_zeros)
    return {
        name: np.asarray(out_arrs[i]).reshape(NCORES, *out_avals[i].shape)
        for i, name in enumerate(out_names)
    }


def kernel(z, codebook):
    outs = _run(_prep_in_maps(z, codebook))
    zq = outs["zq_out"].reshape(B, L, D)
    idx = outs["idx_out"].reshape(B, L).astype(np.int32)
    return zq, idx


if __name__ == "__main__":
    rng = np.random.default_rng(0)
    z = rng.standard_normal((B, L, D), dtype=np.float32)
    cb = rng.standard_normal((K, D), dtype=np.float32)
    zq, idx = kernel(z, cb)
    flat = z.reshape(-1, D)
    d = (flat * flat).sum(1)[:, None] + (cb * cb).sum(1)[None, :] \
        - 2.0 * flat @ cb.T
    eidx = d.argmin(1)
    print("idx mismatches:", (idx.reshape(-1) != eidx).sum(), "/", B * L)
    print("zq maxerr:", np.abs(zq.reshape(-1, D) - cb[eidx]).max())


# revision 20
# speedup vs baseline: 441.5083x; 441.5083x over previous
"""VQ codebook quantizer for Trainium2, data-parallel across 8 NeuronCores.

Problem: z [4, 2048, 512] f32, codebook [8192, 512] f32 ->
  z_q = codebook[argmin_k ||z - c_k||^2]  (shape [4, 2048, 512]),
  indices [4, 2048] int32.

Strategy (per spec sharding hint): shard z rows (8192 total) across 8 cores
(1024 rows each); replicate the codebook. Per core:
  - distances via argmax of s[i,k] = z_i . c_k - 0.5||c_k||^2 computed on the
    PE array with an fp16 hi/lo split (3 fp16 matmuls == fp32-grade accuracy
    at 1 cycle/row instead of fp32's 4 cycles/row)
  - bias add + per-512-chunk top-8 (max8/max_index) on the vector engine
  - final argmax combine across chunks, gather z_q rows with indirect DMA.
"""

import numpy as np

import jax
import concourse.bacc as bacc
import concourse.bass as bass
import concourse.mybir as mybir
import concourse.tile as tile

B, L, D, K = 4, 2048, 512, 8192
NCORES = 8
M = (B * L) // NCORES      # 1024 z-rows per core
P = 128
NZT = M // P               # 8 z-tiles per core
KCH = 512                  # k-chunk width (one PSUM bank of fp32)
NKC = K // KCH             # 16 k-chunks
NDC = D // P               # 4 d-chunks (contraction)

f16 = mybir.dt.float16
f32 = mybir.dt.float32
u16 = mybir.dt.uint16
i32 = mybir.dt.int32


def _build_nc(repeat=1, variant="full"):
    nc = bacc.Bacc(None)
    # transposed z shard, fp16 hi/lo: [D, M]
    zh = nc.dram_tensor("zh", [D, M], f16, kind="ExternalInput")
    zl = nc.dram_tensor("zl", [D, M], f16, kind="ExternalInput")
    # transposed codebook, fp16 hi/lo: [D, K]
    ch = nc.dram_tensor("ch", [D, K], f16, kind="ExternalInput")
    cl = nc.dram_tensor("cl", [D, K], f16, kind="ExternalInput")
    # -0.5*||c_k||^2 broadcast to all partitions: [P, K]
    bias = nc.dram_tensor("bias", [P, K], f32, kind="ExternalInput")
    # natural codebook (gather source): [K, D]
    cb = nc.dram_tensor("cb", [K, D], f32, kind="ExternalInput")
    # per-partition iota 0..127 along free dim: [P, P]
    iota = nc.dram_tensor("iota", [P, P], f32, kind="ExternalInput")
    # two-phase extras: fp16 hi/lo of -0.5||c||^2 as two contract rows, the
    # augmented z rows [z, 1, 0...] and augmented codebook [c, -0.5||c||^2, 0...]
    AUG = 520
    bias2 = nc.dram_tensor("bias2", [2, K], f16, kind="ExternalInput")
    za = nc.dram_tensor("za", [M, AUG], f32, kind="ExternalInput")
    cba = nc.dram_tensor("cba", [K, AUG], f32, kind="ExternalInput")

    idx_out = nc.dram_tensor("idx_out", [M, 1], i32, kind="ExternalOutput")
    zq_out = nc.dram_tensor("zq_out", [M, D], f32, kind="ExternalOutput")

    with tile.TileContext(nc) as tc:
        with (
            tc.tile_pool(name="resident", bufs=1) as rp,
            tc.tile_pool(name="cbchunk", bufs=3) as cp,
            tc.tile_pool(name="scores", bufs=8) as sp,
            tc.tile_pool(name="small", bufs=2) as mp,
            tc.tile_pool(name="psum", bufs=8, space="PSUM") as ps,
        ):
            # ---- resident loads ----
            zh_sb = rp.tile([P, NDC * M], f16)   # d-chunk c at cols [c*M, (c+1)*M)
            iota_sb = rp.tile([P, P], f32)
            nc.sync.dma_start(
                zh_sb[:].rearrange("p (c m) -> p c m", c=NDC),
                zh[:].rearrange("(c p) m -> p c m", p=P),
            )
            nc.sync.dma_start(iota_sb[:], iota[:])

            # resident codebook hi half: d-chunk c at cols [c*K, (c+1)*K)
            ch_res = rp.tile([P, NDC * K], f16)
            for c in range(NDC):
                nc.sync.dma_start(
                    ch_res[:, c * K:(c + 1) * K], ch[c * P:(c + 1) * P, :]
                )

            zl_sb = cl_res = cmax8_all = cidx8_all = None
            if variant != "tp":
                zl_sb = rp.tile([P, NDC * M], f16)
                nc.sync.dma_start(
                    zl_sb[:].rearrange("p (c m) -> p c m", c=NDC),
                    zl[:].rearrange("(c p) m -> p c m", p=P),
                )
                cl_res = rp.tile([P, NDC * K], f16)
                for c in range(NDC):
                    nc.sync.dma_start(
                        cl_res[:, c * K:(c + 1) * K], cl[c * P:(c + 1) * P, :]
                    )
                # per-(z-tile) chunk stats, laid side by side:
                # cmax8_all[:, i*P + j*8 : +8] = top-8 of (tile i, chunk j)
                cmax8_all = rp.tile([P, NZT * P], f32)
                cidx8_all = rp.tile([P, NZT * P], u16)

            import contextlib
            loop_cm = (
                tc.For_i(0, repeat, 1) if repeat > 1 else contextlib.nullcontext()
            )
            if variant == "tp":
                bias2_sb = rp.tile([2, K], f16)
                nc.sync.dma_start(bias2_sb[:], bias2[:])
                ones2 = rp.tile([2, P], f16)
                nc.vector.memset(ones2[:], 1.0)
                with loop_cm:
                    _emit_tp_body(
                        nc, tc, sp, mp, ps, zh_sb, iota_sb, ch_res,
                        bias2_sb, ones2, za, cba, cb, idx_out, zq_out,
                    )
            else:
                with loop_cm:
                    _emit_body(
                        nc, tc, cp, sp, mp, ps,
                        zh_sb, zl_sb, iota_sb, cmax8_all, cidx8_all,
                        ch_res, cl_res, bias, cb, idx_out, zq_out, variant,
                    )
    nc.finalize()
    return nc


def _emit_tp_body(nc, tc, sp, mp, ps, zh_sb, iota_sb, ch_res,
                  bias2_sb, ones2, za, cba, cb, idx_out, zq_out):
    AUG = 520
    NCAND = 8
    for i in range(NZT):
        scores = sp.tile([P, K], f32, tag="scores", bufs=2)
        za_t = mp.tile([P, AUG], f32, tag="za")
        nc.sync.dma_start(za_t[:], za[i * P:(i + 1) * P, :])
        for j in range(NKC):
            S = ps.tile([P, KCH], f32, tag="S")
            # bias matmul first (cheap ldweights), then the 4 coarse chunks
            nc.tensor.matmul(
                S[:], lhsT=ones2[:], rhs=bias2_sb[:, j * KCH:(j + 1) * KCH],
                start=True, stop=False,
            )
            for d in range(NDC):
                nc.tensor.matmul(
                    S[:],
                    lhsT=zh_sb[:, d * M + i * P:d * M + (i + 1) * P],
                    rhs=ch_res[:, d * K + j * KCH:d * K + (j + 1) * KCH],
                    start=False, stop=(d == NDC - 1),
                )
            nc.scalar.copy(scores[:, j * KCH:(j + 1) * KCH], S[:])

        g8 = mp.tile([P, 8], f32, tag="g8")
        p8 = mp.tile([P, 8], mybir.dt.uint16, tag="p8")
        nc.vector.max(out=g8[:], in_=scores[:])
        nc.vector.max_index(out=p8[:], in_max=g8[:], in_values=scores[:])
        candf = mp.tile([P, NCAND], f32, tag="candf")
        nc.vector.tensor_copy(candf[:], p8[:, :NCAND])
        candi = mp.tile([P, NCAND], i32, tag="candi")
        nc.vector.tensor_copy(candi[:], p8[:, :NCAND])

        g_sb = mp.tile([P, NCAND * AUG], f32, tag="gsb", bufs=1)
        for c in range(NCAND):
            nc.gpsimd.indirect_dma_start(
                out=g_sb[:, c * AUG:(c + 1) * AUG],
                out_offset=None,
                in_=cba[:],
                in_offset=bass.IndirectOffsetOnAxis(ap=candi[:, c:c + 1], axis=0),
            )
        junk = mp.tile([P, AUG], f32, tag="junk")
        s8 = mp.tile([P, NCAND], f32, tag="s8")
        for c in range(NCAND):
            nc.vector.tensor_tensor(
                out=junk[:], in0=g_sb[:, c * AUG:(c + 1) * AUG], in1=za_t[:],
                op=mybir.AluOpType.mult,
            )
            nc.vector.tensor_reduce(
                out=s8[:, c:c + 1], in_=junk[:], axis=mybir.AxisListType.X,
                op=mybir.AluOpType.add,
            )
        fx8 = mp.tile([P, 8], f32, tag="fx8")
        fp8 = mp.tile([P, 8], mybir.dt.uint16, tag="fp8")
        nc.vector.max(out=fx8[:], in_=s8[:])
        nc.vector.max_index(out=fp8[:], in_max=fx8[:], in_values=s8[:])
        fposf = mp.tile([P, 1], f32, tag="fposf")
        nc.vector.tensor_copy(fposf[:], fp8[:, 0:1])
        mask8 = mp.tile([P, NCAND], f32, tag="mask8")
        nc.vector.tensor_scalar(
            out=mask8[:], in0=iota_sb[:, :NCAND], scalar1=fposf[:, 0:1],
            scalar2=None, op0=mybir.AluOpType.is_equal,
        )
        picked = mp.tile([P, NCAND], f32, tag="picked8")
        nc.vector.tensor_tensor(
            out=picked[:], in0=mask8[:], in1=candf[:], op=mybir.AluOpType.mult
        )
        fidx = mp.tile([P, 1], f32, tag="fidx")
        nc.vector.tensor_reduce(
            out=fidx[:], in_=picked[:], axis=mybir.AxisListType.X,
            op=mybir.AluOpType.max,
        )
        idx_i32 = mp.tile([P, 1], i32, tag="idxf")
        nc.vector.tensor_copy(idx_i32[:], fidx[:])
        zq_sb = mp.tile([P, D], f32, tag="zqf")
        nc.gpsimd.indirect_dma_start(
            out=zq_sb[:], out_offset=None, in_=cb[:],
            in_offset=bass.IndirectOffsetOnAxis(ap=idx_i32[:, :1], axis=0),
        )
        nc.sync.dma_start(idx_out[i * P:(i + 1) * P, :], idx_i32[:])
        nc.sync.dma_start(zq_out[i * P:(i + 1) * P, :], zq_sb[:])


def _emit_body(nc, tc, cp, sp, mp, ps, zh_sb, zl_sb, iota_sb,
               cmax8_all, cidx8_all, ch_res, cl_res, bias, cb,
               idx_out, zq_out, variant):
    if True:
        if True:
            # ---- main loop: k-chunks outer, z-tiles inner ----
            for j in range(NKC):
                ks = slice(j * KCH, (j + 1) * KCH)
                bias_t = cp.tile([P, KCH], f32, tag="bias")
                nc.sync.dma_start(bias_t[:], bias[:, ks])

                def cast(ap):
                    return (
                        ap.bitcast(mybir.dt.bfloat16) if variant == "pebf" else ap
                    )

                for i in range(NZT):
                    S = ps.tile([P, KCH], f32, tag="S")
                    nmm = 0
                    # group by stationary operand so weight loads amortize
                    for d in range(NDC):
                        zslice = slice(d * M + i * P, d * M + (i + 1) * P)
                        cslice = slice(d * K + j * KCH, d * K + (j + 1) * KCH)
                        for ct in (ch_res, cl_res):
                            nmm += 1
                            nc.tensor.matmul(
                                S[:],
                                lhsT=cast(zh_sb[:, zslice]),
                                rhs=cast(ct[:, cslice]),
                                start=(nmm == 1),
                                stop=False,
                            )
                    for d in range(NDC):
                        zslice = slice(d * M + i * P, d * M + (i + 1) * P)
                        cslice = slice(d * K + j * KCH, d * K + (j + 1) * KCH)
                        nmm += 1
                        nc.tensor.matmul(
                            S[:],
                            lhsT=cast(zl_sb[:, zslice]),
                            rhs=cast(ch_res[:, cslice]),
                            start=False,
                            stop=(nmm == 3 * NDC),
                        )
                    if variant in ("pe", "pebf"):
                        # tiny consumer to keep the accumulation live
                        sc = sp.tile([P, KCH], f32, tag="sc")
                        nc.vector.tensor_tensor(
                            out=sc[:, :8], in0=S[:, :8], in1=bias_t[:, :8],
                            op=mybir.AluOpType.add,
                        )
                        nc.vector.max(
                            out=cmax8_all[:, i * P + j * 8:i * P + j * 8 + 8],
                            in_=sc[:, :8],
                        )
                        continue
                    sc = sp.tile([P, KCH], f32, tag="sc")
                    nc.vector.tensor_tensor(
                        out=sc[:], in0=S[:], in1=bias_t[:], op=mybir.AluOpType.add
                    )
                    slot = slice(i * P + j * 8, i * P + j * 8 + 8)
                    nc.vector.max(out=cmax8_all[:, slot], in_=sc[:])
                    nc.vector.max_index(
                        out=cidx8_all[:, slot], in_max=cmax8_all[:, slot],
                        in_values=sc[:],
                    )
            if variant in ("pe", "pebf"):
                return

            # ---- per-z-tile combine + gather ----
            for i in range(NZT):
                tslot = slice(i * P, (i + 1) * P)
                g8 = mp.tile([P, 8], f32, tag="g8")
                p8 = mp.tile([P, 8], u16, tag="p8")
                nc.vector.max(out=g8[:], in_=cmax8_all[:, tslot])
                nc.vector.max_index(
                    out=p8[:], in_max=g8[:], in_values=cmax8_all[:, tslot]
                )
                p0f = mp.tile([P, 1], f32, tag="p0f")
                nc.vector.tensor_copy(p0f[:], p8[:, 0:1])
                cidxf = mp.tile([P, P], f32, tag="cidxf")
                nc.vector.tensor_copy(cidxf[:], cidx8_all[:, tslot])
                mask = mp.tile([P, P], f32, tag="mask")
                nc.vector.tensor_scalar(
                    out=mask[:], in0=iota_sb[:], scalar1=p0f[:, 0:1], scalar2=None,
                    op0=mybir.AluOpType.is_equal,
                )
                picked = mp.tile([P, P], f32, tag="picked")
                nc.vector.tensor_tensor(
                    out=picked[:], in0=mask[:], in1=cidxf[:],
                    op=mybir.AluOpType.mult,
                )
                inch = mp.tile([P, 1], f32, tag="inch")
                nc.vector.tensor_reduce(
                    out=inch[:], in_=picked[:], axis=mybir.AxisListType.X,
                    op=mybir.AluOpType.max,
                )
                # final = p0*64 + inchunk  (p0 = 8*j  ->  p0*64 = j*512)
                fin = mp.tile([P, 1], f32, tag="fin")
                nc.vector.tensor_scalar(
                    out=fin[:], in0=p0f[:], scalar1=64.0, scalar2=None,
                    op0=mybir.AluOpType.mult,
                )
                fin2 = mp.tile([P, 1], f32, tag="fin2")
                nc.vector.tensor_tensor(
                    out=fin2[:], in0=fin[:], in1=inch[:], op=mybir.AluOpType.add
                )
                idx_i32 = mp.tile([P, 1], i32, tag="idx")
                nc.vector.tensor_copy(idx_i32[:], fin2[:])

                zq_sb = mp.tile([P, D], f32, tag="zq")
                nc.gpsimd.indirect_dma_start(
                    out=zq_sb[:],
                    out_offset=None,
                    in_=cb[:],
                    in_offset=bass.IndirectOffsetOnAxis(ap=idx_i32[:, :1], axis=0),
                )
                nc.sync.dma_start(idx_out[i * P:(i + 1) * P, :], idx_i32[:])
                nc.sync.dma_start(zq_out[i * P:(i + 1) * P, :], zq_sb[:])


_CACHE = {}


def _get_compiled():
    """Build the Bass module and the sharded PJRT callable once per process."""
    if "fn" in _CACHE:
        return _CACHE["fn"]

    from jax.sharding import Mesh, PartitionSpec
    from jax.experimental.shard_map import shard_map
    from concourse.bass2jax import (
        _bass_exec_p, install_neuronx_cc_hook, partition_id_tensor,
    )

    nc = _build_nc(variant="tp")
    install_neuronx_cc_hook()

    in_names = []
    out_names = []
    out_avals = []
    for alloc in nc.m.functions[0].allocations:
        if not isinstance(alloc, mybir.MemoryLocationSet):
            continue
        name = alloc.memorylocations[0].name
        if alloc.kind == "ExternalInput":
            if nc.partition_id_tensor and name == nc.partition_id_tensor.name:
                continue
            in_names.append(name)
        elif alloc.kind == "ExternalOutput":
            out_names.append(name)
            out_avals.append(
                jax.core.ShapedArray(
                    tuple(alloc.tensor_shape), mybir.dt.np(alloc.dtype)
                )
            )
    n_params = len(in_names)
    n_outs = len(out_names)
    all_names = in_names + out_names
    if nc.partition_id_tensor is not None:
        all_names = all_names + [nc.partition_id_tensor.name]

    def _body(*args):
        operands = list(args)
        if nc.partition_id_tensor is not None:
            operands.append(partition_id_tensor())
        outs = _bass_exec_p.bind(
            *operands,
            out_avals=tuple(out_avals),
            in_names=tuple(all_names),
            out_names=tuple(out_names),
            lowering_input_output_aliases=(),
            sim_require_finite=True,
            sim_require_nnan=True,
            nc=nc,
        )
        return tuple(outs)

    devices = jax.devices()[:NCORES]
    mesh = Mesh(np.asarray(devices), ("core",))
    donate = tuple(range(n_params, n_params + n_outs))
    sharded = jax.jit(
        shard_map(
            _body, mesh=mesh,
            in_specs=(PartitionSpec("core"),) * (n_params + n_outs),
            out_specs=(PartitionSpec("core"),) * n_outs,
            check_rep=False,
        ),
        donate_argnums=donate,
        keep_unused=True,
    )
    _CACHE["fn"] = (sharded, in_names, out_names, out_avals)
    return _CACHE["fn"]


def _prep_in_maps(z, codebook):
    z = np.ascontiguousarray(np.asarray(z, dtype=np.float32))
    codebook = np.ascontiguousarray(np.asarray(codebook, dtype=np.float32))
    flat = z.reshape(B * L, D)

    cbT = np.ascontiguousarray(codebook.T)                     # [D, K]
    ch = cbT.astype(np.float16)
    cl = (cbT - ch.astype(np.float32)).astype(np.float16)
    cc = (codebook.astype(np.float64) ** 2).sum(1)
    bias = np.ascontiguousarray(
        np.broadcast_to((-0.5 * cc).astype(np.float32), (P, K))
    )
    iota = np.ascontiguousarray(
        np.broadcast_to(np.arange(P, dtype=np.float32), (P, P))
    )

    # two-phase extras
    AUG = 520
    nb = (-0.5 * cc).astype(np.float32)
    bh = nb.astype(np.float16)
    bl = (nb - bh.astype(np.float32)).astype(np.float16)
    bias2 = np.stack([bh, bl])                                 # [2, K]
    cba = np.zeros((K, AUG), np.float32)
    cba[:, :D] = codebook
    cba[:, D] = nb

    in_maps = []
    for c in range(NCORES):
        shard = flat[c * M:(c + 1) * M]                        # [M, D]
        zT = np.ascontiguousarray(shard.T)                     # [D, M]
        zh = zT.astype(np.float16)
        zl = (zT - zh.astype(np.float32)).astype(np.float16)
        za = np.zeros((M, AUG), np.float32)
        za[:, :D] = shard
        za[:, D] = 1.0
        in_maps.append(dict(
            zh=zh, zl=zl, ch=ch, cl=cl, bias=bias, cb=codebook, iota=iota,
            bias2=bias2, za=za, cba=cba,
        ))
    return in_maps


def _run(in_maps):
    sharded, in_names, out_names, out_avals = _get_compiled()
    concat_in = [
        np.concatenate([in_maps[c][name] for c in range(NCORES)], axis=0)
        for name in in_names
    ]
    concat_zeros = [
        np.zeros((NCORES * a.shape[0], *a.shape[1:]), a.dtype) for a in out_avals
    ]
    out_arrs = sharded(*concat_in, *concat_zeros)
    return {
        name: np.asarray(out_arrs[i]).reshape(NCORES, *out_avals[i].shape)
        for i, name in enumerate(out_names)
    }


def kernel(z, codebook):
    outs = _run(_prep_in_maps(z, codebook))
    zq = outs["zq_out"].reshape(B, L, D)
    idx = outs["idx_out"].reshape(B, L).astype(np.int32)
    return zq, idx


if __name__ == "__main__":
    rng = np.random.default_rng(0)
    z = rng.standard_normal((B, L, D), dtype=np.float32)
    cb = rng.standard_normal((K, D), dtype=np.float32)
    zq, idx = kernel(z, cb)
    flat = z.reshape(-1, D)
    d = (flat * flat).sum(1)[:, None] + (cb * cb).sum(1)[None, :] \
        - 2.0 * flat @ cb.T
    eidx = d.argmin(1)
    print("idx mismatches:", (idx.reshape(-1) != eidx).sum(), "/", B * L)
    print("zq maxerr:", np.abs(zq.reshape(-1, D) - cb[eidx]).max())


# revision 22
# speedup vs baseline: 758.4225x; 1.7178x over previous
"""VQ codebook quantizer for Trainium2, data-parallel across 8 NeuronCores.

Problem: z [4, 2048, 512] f32, codebook [8192, 512] f32 ->
  z_q = codebook[argmin_k ||z - c_k||^2]  (shape [4, 2048, 512]),
  indices [4, 2048] int32.

Strategy (per spec sharding hint): shard z rows (8192 total) across 8 cores
(1024 rows each); replicate the codebook. Per core:
  - distances via argmax of s[i,k] = z_i . c_k - 0.5||c_k||^2 computed on the
    PE array with an fp16 hi/lo split (3 fp16 matmuls == fp32-grade accuracy
    at 1 cycle/row instead of fp32's 4 cycles/row)
  - bias add + per-512-chunk top-8 (max8/max_index) on the vector engine
  - final argmax combine across chunks, gather z_q rows with indirect DMA.
"""

import numpy as np

import jax
import concourse.bacc as bacc
import concourse.bass as bass
import concourse.mybir as mybir
import concourse.tile as tile

B, L, D, K = 4, 2048, 512, 8192
NCORES = 8
M = (B * L) // NCORES      # 1024 z-rows per core
P = 128
NZT = M // P               # 8 z-tiles per core
KCH = 512                  # k-chunk width (one PSUM bank of fp32)
NKC = K // KCH             # 16 k-chunks
NDC = D // P               # 4 d-chunks (contraction)

f16 = mybir.dt.float16
f32 = mybir.dt.float32
u16 = mybir.dt.uint16
i32 = mybir.dt.int32


def _build_nc(repeat=1, variant="full"):
    nc = bacc.Bacc(None)
    # transposed z shard, fp16 hi/lo: [D, M]
    zh = nc.dram_tensor("zh", [D, M], f16, kind="ExternalInput")
    zl = nc.dram_tensor("zl", [D, M], f16, kind="ExternalInput")
    # transposed codebook, fp16 hi/lo: [D, K]
    ch = nc.dram_tensor("ch", [D, K], f16, kind="ExternalInput")
    cl = nc.dram_tensor("cl", [D, K], f16, kind="ExternalInput")
    # -0.5*||c_k||^2 broadcast to all partitions: [P, K]
    bias = nc.dram_tensor("bias", [P, K], f32, kind="ExternalInput")
    # natural codebook (gather source): [K, D]
    cb = nc.dram_tensor("cb", [K, D], f32, kind="ExternalInput")
    # per-partition iota 0..127 along free dim: [P, P]
    iota = nc.dram_tensor("iota", [P, P], f32, kind="ExternalInput")
    # two-phase extras: fp16 hi/lo of -0.5||c||^2 as two contract rows, the
    # augmented z rows [z, 1, 0...] and augmented codebook [c, -0.5||c||^2, 0...]
    AUG = 520
    bias2 = nc.dram_tensor("bias2", [2, K], f16, kind="ExternalInput")
    za = nc.dram_tensor("za", [M, AUG], f32, kind="ExternalInput")
    cba = nc.dram_tensor("cba", [K, AUG], f32, kind="ExternalInput")

    idx_out = nc.dram_tensor("idx_out", [M, 1], i32, kind="ExternalOutput")
    zq_out = nc.dram_tensor("zq_out", [M, D], f32, kind="ExternalOutput")

    with tile.TileContext(nc) as tc:
        with (
            tc.tile_pool(name="resident", bufs=1) as rp,
            tc.tile_pool(name="cbchunk", bufs=3) as cp,
            tc.tile_pool(name="scores", bufs=8) as sp,
            tc.tile_pool(name="small", bufs=2) as mp,
            tc.tile_pool(name="psum", bufs=8, space="PSUM") as ps,
        ):
            # ---- resident loads ----
            zh_sb = rp.tile([P, NDC * M], f16)   # d-chunk c at cols [c*M, (c+1)*M)
            iota_sb = rp.tile([P, P], f32)
            nc.sync.dma_start(
                zh_sb[:].rearrange("p (c m) -> p c m", c=NDC),
                zh[:].rearrange("(c p) m -> p c m", p=P),
            )
            nc.sync.dma_start(iota_sb[:], iota[:])

            # resident codebook hi half: d-chunk c at cols [c*K, (c+1)*K)
            ch_res = rp.tile([P, NDC * K], f16)
            for c in range(NDC):
                nc.sync.dma_start(
                    ch_res[:, c * K:(c + 1) * K], ch[c * P:(c + 1) * P, :]
                )

            zl_sb = cl_res = cmax8_all = cidx8_all = None
            if variant != "tp":
                zl_sb = rp.tile([P, NDC * M], f16)
                nc.sync.dma_start(
                    zl_sb[:].rearrange("p (c m) -> p c m", c=NDC),
                    zl[:].rearrange("(c p) m -> p c m", p=P),
                )
                cl_res = rp.tile([P, NDC * K], f16)
                for c in range(NDC):
                    nc.sync.dma_start(
                        cl_res[:, c * K:(c + 1) * K], cl[c * P:(c + 1) * P, :]
                    )
                # per-(z-tile) chunk stats, laid side by side:
                # cmax8_all[:, i*P + j*8 : +8] = top-8 of (tile i, chunk j)
                cmax8_all = rp.tile([P, NZT * P], f32)
                cidx8_all = rp.tile([P, NZT * P], u16)

            import contextlib
            loop_cm = (
                tc.For_i(0, repeat, 1) if repeat > 1 else contextlib.nullcontext()
            )
            if variant == "tp":
                bias2_sb = rp.tile([2, K], f16)
                nc.sync.dma_start(bias2_sb[:], bias2[:])
                ones2 = rp.tile([2, P], f16)
                nc.vector.memset(ones2[:], 1.0)
                with loop_cm:
                    _emit_tp_body(
                        nc, tc, sp, mp, ps, zh_sb, iota_sb, ch_res,
                        bias2_sb, ones2, za, cba, cb, idx_out, zq_out,
                    )
            else:
                with loop_cm:
                    _emit_body(
                        nc, tc, cp, sp, mp, ps,
                        zh_sb, zl_sb, iota_sb, cmax8_all, cidx8_all,
                        ch_res, cl_res, bias, cb, idx_out, zq_out, variant,
                    )
    nc.finalize()
    return nc


def _emit_tp_body(nc, tc, sp, mp, ps, zh_sb, iota_sb, ch_res,
                  bias2_sb, ones2, za, cba, cb, idx_out, zq_out):
    AUG = 520
    NCAND = 4
    for i in range(NZT):
        scores = sp.tile([P, K], f32, tag="scores", bufs=2)
        za_t = mp.tile([P, AUG], f32, tag="za")
        nc.sync.dma_start(za_t[:], za[i * P:(i + 1) * P, :])
        for j in range(NKC):
            S = ps.tile([P, KCH], f32, tag="S")
            # bias matmul first (cheap ldweights), then the 4 coarse chunks
            nc.tensor.matmul(
                S[:], lhsT=ones2[:], rhs=bias2_sb[:, j * KCH:(j + 1) * KCH],
                start=True, stop=False,
            )
            for d in range(NDC):
                nc.tensor.matmul(
                    S[:],
                    lhsT=zh_sb[:, d * M + i * P:d * M + (i + 1) * P],
                    rhs=ch_res[:, d * K + j * KCH:d * K + (j + 1) * KCH],
                    start=False, stop=(d == NDC - 1),
                )
            nc.scalar.copy(scores[:, j * KCH:(j + 1) * KCH], S[:])

        g8 = mp.tile([P, 8], f32, tag="g8")
        p8 = mp.tile([P, 8], mybir.dt.uint16, tag="p8")
        nc.vector.max(out=g8[:], in_=scores[:])
        nc.vector.max_index(out=p8[:], in_max=g8[:], in_values=scores[:])
        candf = mp.tile([P, NCAND], f32, tag="candf")
        nc.vector.tensor_copy(candf[:], p8[:, :NCAND])
        candi = mp.tile([P, NCAND], i32, tag="candi")
        nc.vector.tensor_copy(candi[:], p8[:, :NCAND])

        g_sb = mp.tile([P, NCAND * AUG], f32, tag="gsb", bufs=2)
        for c in range(NCAND):
            nc.gpsimd.indirect_dma_start(
                out=g_sb[:, c * AUG:(c + 1) * AUG],
                out_offset=None,
                in_=cba[:],
                in_offset=bass.IndirectOffsetOnAxis(ap=candi[:, c:c + 1], axis=0),
            )
        junk = mp.tile([P, AUG], f32, tag="junk")
        s8 = mp.tile([P, NCAND], f32, tag="s8")
        for c in range(NCAND):
            nc.vector.tensor_tensor(
                out=junk[:], in0=g_sb[:, c * AUG:(c + 1) * AUG], in1=za_t[:],
                op=mybir.AluOpType.mult,
            )
            nc.vector.tensor_reduce(
                out=s8[:, c:c + 1], in_=junk[:], axis=mybir.AxisListType.X,
                op=mybir.AluOpType.add,
            )
        best = mp.tile([P, 1], f32, tag="best")
        nc.vector.tensor_reduce(
            out=best[:], in_=s8[:], axis=mybir.AxisListType.X,
            op=mybir.AluOpType.max,
        )
        mask8 = mp.tile([P, NCAND], f32, tag="mask8")
        nc.vector.tensor_scalar(
            out=mask8[:], in0=s8[:], scalar1=best[:, 0:1],
            scalar2=None, op0=mybir.AluOpType.is_equal,
        )
        picked = mp.tile([P, NCAND], f32, tag="picked8")
        nc.vector.tensor_tensor(
            out=picked[:], in0=mask8[:], in1=candf[:], op=mybir.AluOpType.mult
        )
        fidx = mp.tile([P, 1], f32, tag="fidx")
        nc.vector.tensor_reduce(
            out=fidx[:], in_=picked[:], axis=mybir.AxisListType.X,
            op=mybir.AluOpType.max,
        )
        idx_i32 = mp.tile([P, 1], i32, tag="idxf")
        nc.vector.tensor_copy(idx_i32[:], fidx[:])
        zq_sb = mp.tile([P, D], f32, tag="zqf")
        nc.gpsimd.indirect_dma_start(
            out=zq_sb[:], out_offset=None, in_=cb[:],
            in_offset=bass.IndirectOffsetOnAxis(ap=idx_i32[:, :1], axis=0),
        )
        nc.sync.dma_start(idx_out[i * P:(i + 1) * P, :], idx_i32[:])
        nc.sync.dma_start(zq_out[i * P:(i + 1) * P, :], zq_sb[:])


def _emit_body(nc, tc, cp, sp, mp, ps, zh_sb, zl_sb, iota_sb,
               cmax8_all, cidx8_all, ch_res, cl_res, bias, cb,
               idx_out, zq_out, variant):
    if True:
        if True:
            # ---- main loop: k-chunks outer, z-tiles inner ----
            for j in range(NKC):
                ks = slice(j * KCH, (j + 1) * KCH)
                bias_t = cp.tile([P, KCH], f32, tag="bias")
                nc.sync.dma_start(bias_t[:], bias[:, ks])

                def cast(ap):
                    return (
                        ap.bitcast(mybir.dt.bfloat16) if variant == "pebf" else ap
                    )

                for i in range(NZT):
                    S = ps.tile([P, KCH], f32, tag="S")
                    nmm = 0
                    # group by stationary operand so weight loads amortize
                    for d in range(NDC):
                        zslice = slice(d * M + i * P, d * M + (i + 1) * P)
                        cslice = slice(d * K + j * KCH, d * K + (j + 1) * KCH)
                        for ct in (ch_res, cl_res):
                            nmm += 1
                            nc.tensor.matmul(
                                S[:],
                                lhsT=cast(zh_sb[:, zslice]),
                                rhs=cast(ct[:, cslice]),
                                start=(nmm == 1),
                                stop=False,
                            )
                    for d in range(NDC):
                        zslice = slice(d * M + i * P, d * M + (i + 1) * P)
                        cslice = slice(d * K + j * KCH, d * K + (j + 1) * KCH)
                        nmm += 1
                        nc.tensor.matmul(
                            S[:],
                            lhsT=cast(zl_sb[:, zslice]),
                            rhs=cast(ch_res[:, cslice]),
                            start=False,
                            stop=(nmm == 3 * NDC),
                        )
                    if variant in ("pe", "pebf"):
                        # tiny consumer to keep the accumulation live
                        sc = sp.tile([P, KCH], f32, tag="sc")
                        nc.vector.tensor_tensor(
                            out=sc[:, :8], in0=S[:, :8], in1=bias_t[:, :8],
                            op=mybir.AluOpType.add,
                        )
                        nc.vector.max(
                            out=cmax8_all[:, i * P + j * 8:i * P + j * 8 + 8],
                            in_=sc[:, :8],
                        )
                        continue
                    sc = sp.tile([P, KCH], f32, tag="sc")
                    nc.vector.tensor_tensor(
                        out=sc[:], in0=S[:], in1=bias_t[:], op=mybir.AluOpType.add
                    )
                    slot = slice(i * P + j * 8, i * P + j * 8 + 8)
                    nc.vector.max(out=cmax8_all[:, slot], in_=sc[:])
                    nc.vector.max_index(
                        out=cidx8_all[:, slot], in_max=cmax8_all[:, slot],
                        in_values=sc[:],
                    )
            if variant in ("pe", "pebf"):
                return

            # ---- per-z-tile combine + gather ----
            for i in range(NZT):
                tslot = slice(i * P, (i + 1) * P)
                g8 = mp.tile([P, 8], f32, tag="g8")
                p8 = mp.tile([P, 8], u16, tag="p8")
                nc.vector.max(out=g8[:], in_=cmax8_all[:, tslot])
                nc.vector.max_index(
                    out=p8[:], in_max=g8[:], in_values=cmax8_all[:, tslot]
                )
                p0f = mp.tile([P, 1], f32, tag="p0f")
                nc.vector.tensor_copy(p0f[:], p8[:, 0:1])
                cidxf = mp.tile([P, P], f32, tag="cidxf")
                nc.vector.tensor_copy(cidxf[:], cidx8_all[:, tslot])
                mask = mp.tile([P, P], f32, tag="mask")
                nc.vector.tensor_scalar(
                    out=mask[:], in0=iota_sb[:], scalar1=p0f[:, 0:1], scalar2=None,
                    op0=mybir.AluOpType.is_equal,
                )
                picked = mp.tile([P, P], f32, tag="picked")
                nc.vector.tensor_tensor(
                    out=picked[:], in0=mask[:], in1=cidxf[:],
                    op=mybir.AluOpType.mult,
                )
                inch = mp.tile([P, 1], f32, tag="inch")
                nc.vector.tensor_reduce(
                    out=inch[:], in_=picked[:], axis=mybir.AxisListType.X,
                    op=mybir.AluOpType.max,
                )
                # final = p0*64 + inchunk  (p0 = 8*j  ->  p0*64 = j*512)
                fin = mp.tile([P, 1], f32, tag="fin")
                nc.vector.tensor_scalar(
                    out=fin[:], in0=p0f[:], scalar1=64.0, scalar2=None,
                    op0=mybir.AluOpType.mult,
                )
                fin2 = mp.tile([P, 1], f32, tag="fin2")
                nc.vector.tensor_tensor(
                    out=fin2[:], in0=fin[:], in1=inch[:], op=mybir.AluOpType.add
                )
                idx_i32 = mp.tile([P, 1], i32, tag="idx")
                nc.vector.tensor_copy(idx_i32[:], fin2[:])

                zq_sb = mp.tile([P, D], f32, tag="zq")
                nc.gpsimd.indirect_dma_start(
                    out=zq_sb[:],
                    out_offset=None,
                    in_=cb[:],
                    in_offset=bass.IndirectOffsetOnAxis(ap=idx_i32[:, :1], axis=0),
                )
                nc.sync.dma_start(idx_out[i * P:(i + 1) * P, :], idx_i32[:])
                nc.sync.dma_start(zq_out[i * P:(i + 1) * P, :], zq_sb[:])


_CACHE = {}


def _get_compiled():
    """Build the Bass module and the sharded PJRT callable once per process."""
    if "fn" in _CACHE:
        return _CACHE["fn"]

    from jax.sharding import Mesh, PartitionSpec
    from jax.experimental.shard_map import shard_map
    from concourse.bass2jax import (
        _bass_exec_p, install_neuronx_cc_hook, partition_id_tensor,
    )

    nc = _build_nc(variant="tp")
    install_neuronx_cc_hook()

    in_names = []
    out_names = []
    out_avals = []
    for alloc in nc.m.functions[0].allocations:
        if not isinstance(alloc, mybir.MemoryLocationSet):
            continue
        name = alloc.memorylocations[0].name
        if alloc.kind == "ExternalInput":
            if nc.partition_id_tensor and name == nc.partition_id_tensor.name:
                continue
            in_names.append(name)
        elif alloc.kind == "ExternalOutput":
            out_names.append(name)
            out_avals.append(
                jax.core.ShapedArray(
                    tuple(alloc.tensor_shape), mybir.dt.np(alloc.dtype)
                )
            )
    n_params = len(in_names)
    n_outs = len(out_names)
    all_names = in_names + out_names
    if nc.partition_id_tensor is not None:
        all_names = all_names + [nc.partition_id_tensor.name]

    def _body(*args):
        operands = list(args)
        if nc.partition_id_tensor is not None:
            operands.append(partition_id_tensor())
        outs = _bass_exec_p.bind(
            *operands,
            out_avals=tuple(out_avals),
            in_names=tuple(all_names),
            out_names=tuple(out_names),
            lowering_input_output_aliases=(),
            sim_require_finite=True,
            sim_require_nnan=True,
            nc=nc,
        )
        return tuple(outs)

    devices = jax.devices()[:NCORES]
    mesh = Mesh(np.asarray(devices), ("core",))
    donate = tuple(range(n_params, n_params + n_outs))
    sharded = jax.jit(
        shard_map(
            _body, mesh=mesh,
            in_specs=(PartitionSpec("core"),) * (n_params + n_outs),
            out_specs=(PartitionSpec("core"),) * n_outs,
            check_rep=False,
        ),
        donate_argnums=donate,
        keep_unused=True,
    )
    _CACHE["fn"] = (sharded, in_names, out_names, out_avals)
    return _CACHE["fn"]


def _prep_in_maps(z, codebook):
    z = np.ascontiguousarray(np.asarray(z, dtype=np.float32))
    codebook = np.ascontiguousarray(np.asarray(codebook, dtype=np.float32))
    flat = z.reshape(B * L, D)

    cbT = np.ascontiguousarray(codebook.T)                     # [D, K]
    ch = cbT.astype(np.float16)
    cl = (cbT - ch.astype(np.float32)).astype(np.float16)
    cc = (codebook.astype(np.float64) ** 2).sum(1)
    bias = np.ascontiguousarray(
        np.broadcast_to((-0.5 * cc).astype(np.float32), (P, K))
    )
    iota = np.ascontiguousarray(
        np.broadcast_to(np.arange(P, dtype=np.float32), (P, P))
    )

    # two-phase extras
    AUG = 520
    nb = (-0.5 * cc).astype(np.float32)
    bh = nb.astype(np.float16)
    bl = (nb - bh.astype(np.float32)).astype(np.float16)
    bias2 = np.stack([bh, bl])                                 # [2, K]
    cba = np.zeros((K, AUG), np.float32)
    cba[:, :D] = codebook
    cba[:, D] = nb

    in_maps = []
    for c in range(NCORES):
        shard = flat[c * M:(c + 1) * M]                        # [M, D]
        zT = np.ascontiguousarray(shard.T)                     # [D, M]
        zh = zT.astype(np.float16)
        zl = (zT - zh.astype(np.float32)).astype(np.float16)
        za = np.zeros((M, AUG), np.float32)
        za[:, :D] = shard
        za[:, D] = 1.0
        in_maps.append(dict(
            zh=zh, zl=zl, ch=ch, cl=cl, bias=bias, cb=codebook, iota=iota,
            bias2=bias2, za=za, cba=cba,
        ))
    return in_maps


def _run(in_maps):
    sharded, in_names, out_names, out_avals = _get_compiled()
    concat_in = [
        np.concatenate([in_maps[c][name] for c in range(NCORES)], axis=0)
        for name in in_names
    ]
    concat_zeros = [
        np.zeros((NCORES * a.shape[0], *a.shape[1:]), a.dtype) for a in out_avals
    ]
    out_arrs = sharded(*concat_in, *concat_zeros)
    return {
        name: np.asarray(out_arrs[i]).reshape(NCORES, *out_avals[i].shape)
        for i, name in enumerate(out_names)
    }


def kernel(z, codebook):
    outs = _run(_prep_in_maps(z, codebook))
    zq = outs["zq_out"].reshape(B, L, D)
    idx = outs["idx_out"].reshape(B, L).astype(np.int32)
    return zq, idx


if __name__ == "__main__":
    rng = np.random.default_rng(0)
    z = rng.standard_normal((B, L, D), dtype=np.float32)
    cb = rng.standard_normal((K, D), dtype=np.float32)
    zq, idx = kernel(z, cb)
    flat = z.reshape(-1, D)
    d = (flat * flat).sum(1)[:, None] + (cb * cb).sum(1)[None, :] \
        - 2.0 * flat @ cb.T
    eidx = d.argmin(1)
    print("idx mismatches:", (idx.reshape(-1) != eidx).sum(), "/", B * L)
    print("zq maxerr:", np.abs(zq.reshape(-1, D) - cb[eidx]).max())
